# revision 7
# baseline (speedup 1.0000x reference)
"""Bahdanau additive attention on 8 Trainium2 NeuronCores — fp8 DoubleRow.

Math (per batch b):
    dec_f  = decoder_hidden @ W_h                     [H]   (host, fp32)
    enc_f  = encoder_outputs[b] @ W_s                 [S, H]
    energy = tanh(dec_f + enc_f) @ v + addmask        [S]
    attn   = softmax(energy)                          [S]
    context= attn @ encoder_outputs[b]                [2H]

Sharding: data-parallel over batch, 8 batches per core, weights replicated.

The big matmul runs in fp8e4 with perf_mode=DoubleRow (2 contraction
tiles per matmul, 2x PE throughput): host pre-scales enc by 2^4 and W_s
by 2^8 (clip +-240), quantizes to e4m3; the tanh activation applies
scale=2^-12 with dec_f (host fp32) as per-partition bias.

fp8 error rescue (zero device cost): the host folds the exact linear
correction into the mask row:
    em = enc@(W_s@v) - 2^-12 * enc8@(W8@v) + mask
so device energy = em + v.tanh(x8) = exact_linear + v.(tanh(x8) - x8),
i.e. the quantization error only enters through d/dx[x - tanh(x)] =
tanh^2(x) (mean ~0.24) instead of tanh'(x) (mean ~0.44). dec_f cancels
identically so it never appears in em.

Engine split (PE is the roofline engine; everything else must fit in
its shadow): the PE does ONLY the DoubleRow mains plus, per batch, 4
cross-partition sum matmuls, 4 K=1 em-row adds and 2 tiny softmax
matmuls. The v-contraction runs as a DVE scalar_tensor_tensor chain
(acc_k = hid_k * v_k + acc_{k-1}, bf16, per-partition scalar), whose
[128,512] result one ones-stationary matmul collapses into the packed
energy PSUM row. Context = one fused scalar_tensor_tensor per e-tile
(product + free-dim accumulate) against a separate UNSCALED bf16 copy
of encT. The last batch reuses the same context path with UNNORMALIZED
exp weights (broadcast via a DRAM scratch row, no wait on the softmax
normalization chain); its ctx_acc columns are rescaled by 1/sum once at
the end.
"""

import numpy as np
import ml_dtypes

import concourse.bacc as bacc
import concourse.mybir as mybir
import concourse.tile as tile
from concourse.bass_utils import run_bass_kernel_spmd

# Problem shapes (hardcoded per contest rules).
B, S, H = 64, 2048, 1024
E = 2 * H            # encoder feature dim
NC = 8               # cores
BPC = B // NC        # batches per core
P = 128              # partitions
ET = E // P          # 16 e-tiles (contraction tiles of main matmul)
EPAIR = ET // 2      # 8 e-tile pairs (DoubleRow processes 2 at once)
KT = H // P          # 8 k-tiles (hidden dim tiles)
SC = S // 512        # 4 s-chunks of 512
NEG_BIG = -1e10

SCALE_ENC = 16.0     # enc pre-scale before e4m3 cast
SCALE_W = 256.0      # W_s pre-scale before e4m3 cast
UNSCALE = 1.0 / (SCALE_ENC * SCALE_W)
F8MAX = 240.0        # TRN e4m3 max normal

F32 = mybir.dt.float32
BF16 = mybir.dt.bfloat16
F8 = mybir.dt.float8e4

_CACHE = {}


def _build():
    nc = bacc.Bacc("TRN2", target_bir_lowering=False, debug=False, num_devices=NC)

    encH_d = nc.dram_tensor("encH", [BPC, E, S], F8, kind="ExternalInput")
    encB_d = nc.dram_tensor("encB", [BPC, E, S], BF16, kind="ExternalInput")
    ws_d = nc.dram_tensor("wsT", [E, H], F8, kind="ExternalInput")
    decf_d = nc.dram_tensor("decf", [P, KT * BPC], F32, kind="ExternalInput")
    v_d = nc.dram_tensor("vv", [P, KT], BF16, kind="ExternalInput")
    em_d = nc.dram_tensor("emask", [BPC, S], BF16, kind="ExternalInput")

    attn_d = nc.dram_tensor("attn", [BPC, S], F32, kind="ExternalOutput")
    ctx_d = nc.dram_tensor("ctxr", [P, BPC * ET], F32, kind="ExternalOutput")

    with tile.TileContext(nc) as tc:
        with (
            tc.tile_pool(name="const", bufs=1) as cpool,
            tc.tile_pool(name="psum_mm", bufs=7, space="PSUM") as mmp,
            tc.tile_pool(name="psum_en", bufs=1, space="PSUM") as enp,
        ):
            # ---- persistent constants ----
            v_sb = cpool.tile([P, KT], BF16)
            v_f32 = cpool.tile([P, KT], F32)
            ws_sb = cpool.tile([P, ET, H], F8)
            decf_sb = cpool.tile([P, KT, BPC], F32)
            one_one = cpool.tile([1, 1], BF16)
            nc.vector.memset(one_one[:], 1.0)
            ones_row = cpool.tile([1, P], F32)
            nc.vector.memset(ones_row[:], 1.0)
            ones_col = cpool.tile([P, 1], BF16)
            nc.vector.memset(ones_col[:], 1.0)
            # energy lives packed on partitions {0,32,64,96} of ONE psum
            # bank; ones4 selects those rows in the cross-partition sum
            # matmul, ssum_t holds the per-chunk exp partial sums.
            ones4 = cpool.tile([97, 1], F32)
            nc.vector.memset(ones4[:], 0.0)
            ssum_t = cpool.tile([97, 1], F32)
            nc.vector.memset(ssum_t[:], 0.0)
            for c in range(SC):
                nc.vector.memset(ones4[32 * c:32 * c + 1, :], 1.0)
            ctx_acc = cpool.tile([P, BPC * ET], F32)
            ctx_last = cpool.tile([P, ET], F32)

            # ---- batch pipeline ----
            with (
                tc.tile_pool(name="encp", bufs=8) as encp,
                tc.tile_pool(name="encbp", bufs=5) as encbp,
                tc.tile_pool(name="work", bufs=2) as wkp,
                tc.tile_pool(name="dscr", bufs=1, space="DRAM") as dscr,
            ):
                # startup: tiny tensors, then just the k=0 columns of W_s so
                # batch 0's first mains aren't gated on the full weight load.
                nc.sync.dma_start(v_sb[:], v_d.ap())
                nc.vector.tensor_copy(v_f32[:], v_sb[:])
                nc.sync.dma_start(decf_sb[:], decf_d.ap())
                ws_ap = ws_d.ap().rearrange("(t p) k -> p t k", p=P)
                nc.sync.dma_start(ws_sb[:, :, 0:P], ws_ap[:, :, 0:P])

                state = {}

                def emit_load(b):
                    quarters = []
                    for q in range(4):
                        encq = encp.tile([P, 4, S], F8, tag="encq", name=f"encq_{b}_{q}")
                        nc.sync.dma_start(
                            encq[:],
                            encH_d.ap()[b, q * 512:(q + 1) * 512, :].rearrange(
                                "(t p) s -> p t s", p=P
                            ),
                        )
                        quarters.append(encq)
                    mask_st = wkp.tile([1, S], BF16, tag="mask", bufs=1, name=f"mask_{b}")
                    nc.sync.dma_start(mask_st[:], em_d.ap()[b:b + 1, :])
                    state[b] = (quarters, mask_st)

                def emit_loadB(b):
                    quarters = []
                    for q in range(4):
                        encq = encbp.tile([P, 4, S], BF16, tag="encb", name=f"encb_{b}_{q}")
                        nc.sync.dma_start(
                            encq[:],
                            encB_d.ap()[b, q * 512:(q + 1) * 512, :].rearrange(
                                "(t p) s -> p t s", p=P
                            ),
                        )
                        quarters.append(encq)
                    state[b, "ctxq"] = quarters

                def emit_energy_tail(b):
                    """Collapse batch b's v-chain results into the packed
                    energy PSUM rows (cross-partition ones-matmul) and add
                    the em row (linear correction + mask). Emitted at the
                    start of batch b+1 so the chain has the whole of batch b
                    to complete on the DVE."""
                    energy_ps = state[b, "energy"]
                    _, mask_st = state[b]
                    for sc in range(SC):
                        nc.tensor.matmul(
                            energy_ps[32 * sc:32 * sc + 1, :],
                            ones_col[:],
                            state[b, "vacc", sc][:],
                            start=True,
                            stop=False,
                            skip_group_check=True,
                            tile_position=(0, 32 * sc),
                        )
                        nc.tensor.matmul(
                            energy_ps[32 * sc:32 * sc + 1, :],
                            one_one[:],
                            mask_st[0:1, sc * 512:(sc + 1) * 512],
                            start=False,
                            stop=True,
                            skip_group_check=True,
                            tile_position=(0, 32 * sc),
                        )

                def emit_exp(b):
                    """Exp over batch b's energy PSUM (4 packed chunk rows)."""
                    energy_ps = state[b, "energy"]
                    exps = wkp.tile([97, 512], F32, tag="exps", bufs=1,
                                    name=f"exps_{b}")
                    for c in range(SC):
                        nc.scalar.activation(
                            exps[32 * c:32 * c + 1, :],
                            energy_ps[32 * c:32 * c + 1, :],
                            mybir.ActivationFunctionType.Exp,
                            accum_out=ssum_t[32 * c:32 * c + 1, :],
                        )
                    state[b, "exp"] = exps

                def emit_sum(b):
                    """Cross-partition exp total + reciprocal for batch b."""
                    sum_ps = mmp.tile([1, 1], F32, tag="mm", name=f"sum_ps_{b}")
                    nc.tensor.matmul(sum_ps[:], ssum_t[:], ones4[:],
                                     start=True, stop=True)
                    sinv = wkp.tile([1, 1], F32, tag="sinv", name=f"sinv_{b}")
                    nc.vector.reciprocal(sinv[:], sum_ps[:])
                    state[b, "sinv"] = sinv

                def emit_post(b, last=False):
                    """Softmax tail + attn broadcast + context for batch b."""
                    exps = state[b, "exp"]
                    sinv = state[b, "sinv"]
                    attn_bc = wkp.tile([P, S], BF16, tag="attn_bc", name=f"attn_bc_{b}")
                    if last:
                        # tail fast path: broadcast UNNORMALIZED bf16 exps via
                        # a DRAM scratch row with HWDGE; context is computed
                        # unnormalized into ctx_last and rescaled at the end.
                        exps_bf = wkp.tile([97, 512], BF16, tag="exps_bf", bufs=1,
                                           name=f"exps_bf_{b}")
                        nc.vector.tensor_copy(exps_bf[:], exps[:])
                        bsc = dscr.tile([1, S], BF16, name=f"bsc_{b}")
                        for c in range(SC):
                            nc.sync.dma_start(
                                bsc[0:1, c * 512:(c + 1) * 512],
                                exps_bf[32 * c:32 * c + 1, :],
                            )
                        nc.sync.dma_start(
                            attn_bc[:], bsc[0:1, :].broadcast_to((P, S))
                        )
                    # broadcast 1/sum to all partitions with a K=1 matmul
                    sb_ps = mmp.tile([P, 1], F32, tag="mm", name=f"sb_ps_{b}")
                    nc.tensor.matmul(sb_ps[:], ones_row[:], sinv[:],
                                     start=True, stop=True)
                    sinv_all = wkp.tile([P, 1], F32, tag="sinv_all",
                                        name=f"sinv_all_{b}")
                    nc.vector.tensor_copy(sinv_all[:], sb_ps[:])
                    attn_row = wkp.tile([97, 512], F32, tag="attn_row", bufs=1,
                                        name=f"attn_row_{b}")
                    nc.vector.tensor_scalar_mul(attn_row[:], exps[:],
                                                sinv_all[0:97, :])
                    for c in range(SC):
                        nc.sync.dma_start(
                            attn_d.ap()[b:b + 1, c * 512:(c + 1) * 512],
                            attn_row[32 * c:32 * c + 1, :],
                        )
                    if not last:
                        # pipelined path: broadcast the normalized attn row
                        # from its DRAM output slot (SWDGE: cast in flight)
                        nc.gpsimd.dma_start(
                            attn_bc[:], attn_d.ap()[b:b + 1, :].broadcast_to((P, S))
                        )

                    # context: one fused multiply + free-dim accumulate per
                    # e-tile over the bf16 encT tiles
                    ctxq = state[b, "ctxq"]
                    for e in range(ET):
                        q, qt = divmod(e, 4)
                        scr = wkp.tile([P, S], BF16, tag="scr", bufs=2,
                                       name=f"scr_{b}_{e}")
                        if last:
                            acc = ctx_last[:, e:e + 1]
                        else:
                            acc = ctx_acc[:, b * ET + e:b * ET + e + 1]
                        nc.vector.scalar_tensor_tensor(
                            scr[:],
                            ctxq[q][:, qt, :],
                            1.0,
                            attn_bc[:],
                            mybir.AluOpType.mult,
                            mybir.AluOpType.mult,
                            accum_out=acc,
                        )
                    if last:
                        # fold 1/sum into the unnormalized last-batch context
                        nc.vector.tensor_scalar_mul(
                            ctx_acc[:, b * ET:(b + 1) * ET], ctx_last[:],
                            sinv_all[:],
                        )

                # batch 0's data next on the DMA queue, then the rest of W_s
                emit_load(0)
                nc.sync.dma_start(ws_sb[:, :, P:H], ws_ap[:, :, P:H])
                emit_load(1)
                for b in range(BPC):
                    quarters, mask_st = state[b]
                    if b > 0:
                        emit_energy_tail(b - 1)
                        emit_exp(b - 1)
                    energy_ps = enp.tile([97, 512], F32, tag="energy", name=f"energy_{b}")
                    state[b, "energy"] = energy_ps

                    # main matmul: fp8 DoubleRow over e-tile pairs; each
                    # stationary W_s pair-tile serves all 4 s-chunks. The
                    # v-chain for k runs on the DVE one k-iteration later so
                    # it never waits on the tanh that produces its input.
                    hid_prev = None
                    for k in range(KT):
                        if k == 1 and b > 0:
                            emit_sum(b - 1)
                        if k == 2 and b > 0:
                            emit_post(b - 1)
                        if k == 3:
                            emit_loadB(b)
                        if k == 4 and b + 2 < BPC:
                            emit_load(b + 2)
                        pss = []
                        for sc in range(SC):
                            ps = mmp.tile([P, 512], F32, tag="mm",
                                          name=f"ps_{b}_{k}_{sc}")
                            pss.append(ps)
                        for ep in range(EPAIR):
                            q, qt = divmod(2 * ep, 4)
                            for sc in range(SC):
                                nc.tensor.matmul(
                                    pss[sc][:],
                                    ws_sb[:, 2 * ep:2 * ep + 2, k * P:(k + 1) * P],
                                    quarters[q][:, qt:qt + 2, sc * 512:(sc + 1) * 512],
                                    start=(ep == 0),
                                    stop=(ep == EPAIR - 1),
                                    perf_mode=mybir.MatmulPerfMode.DoubleRow,
                                )
                        # v-chain steps for k-1's hids (DVE)
                        if hid_prev is not None:
                            for sc in range(SC):
                                vacc = wkp.tile([P, 512], BF16, tag="vacc",
                                                bufs=8, name=f"vacc_{b}_{k-1}_{sc}")
                                if k == 1:
                                    nc.vector.tensor_scalar_mul(
                                        vacc[:], hid_prev[sc][:],
                                        v_f32[:, k - 1:k])
                                else:
                                    nc.vector.scalar_tensor_tensor(
                                        vacc[:], hid_prev[sc][:],
                                        v_f32[:, k - 1:k],
                                        state[b, "vacc", sc][:],
                                        mybir.AluOpType.mult,
                                        mybir.AluOpType.add,
                                    )
                                state[b, "vacc", sc] = vacc
                        hids = []
                        for sc in range(SC):
                            hid = wkp.tile([P, 512], BF16, tag="hid", bufs=8,
                                           name=f"hid_{b}_{k}_{sc}")
                            nc.scalar.activation(
                                hid[:],
                                pss[sc][:],
                                mybir.ActivationFunctionType.Tanh,
                                bias=decf_sb[:, k, b:b + 1],
                                scale=UNSCALE,
                            )
                            hids.append(hid)
                        hid_prev = hids
                    # final v-chain steps (k = KT-1)
                    for sc in range(SC):
                        vacc = wkp.tile([P, 512], BF16, tag="vacc", bufs=8,
                                        name=f"vacc_{b}_{KT-1}_{sc}")
                        nc.vector.scalar_tensor_tensor(
                            vacc[:], hid_prev[sc][:], v_f32[:, KT - 1:KT],
                            state[b, "vacc", sc][:],
                            mybir.AluOpType.mult, mybir.AluOpType.add,
                        )
                        state[b, "vacc", sc] = vacc
                emit_energy_tail(BPC - 1)
                emit_exp(BPC - 1)
                emit_sum(BPC - 1)
                emit_post(BPC - 1, last=True)

            nc.sync.dma_start(ctx_d.ap()[:], ctx_acc[:])

    nc.compile()
    return nc


def _prep_inputs(decoder_hidden, encoder_outputs, src_mask, W_h, W_s, v):
    bf = ml_dtypes.bfloat16
    f8 = ml_dtypes.float8_e4m3

    enc = np.asarray(encoder_outputs, np.float32)          # [B, S, E]
    W_s = np.asarray(W_s, np.float32)
    v_b = np.asarray(v, np.float32).astype(bf).astype(np.float32)

    # quantized weights + linear-correction vectors
    W8 = np.clip(W_s * SCALE_W, -F8MAX, F8MAX).astype(f8)
    u = W_s @ v_b                                          # [E] exact
    u8 = W8.astype(np.float32) @ v_b                       # [E] of quantized W

    decf = np.asarray(decoder_hidden, np.float32) @ np.asarray(W_h, np.float32)

    vv = np.ascontiguousarray(v_b.astype(bf).reshape(KT, P).T)
    maskadd = np.where(np.asarray(src_mask) == 0, np.float32(NEG_BIG),
                       np.float32(0.0))

    in_maps = []
    for c in range(NC):
        lo, hi = c * BPC, (c + 1) * BPC
        encT = np.ascontiguousarray(enc[lo:hi].transpose(0, 2, 1))  # [BPC,E,S]
        encH = np.clip(encT * np.float32(SCALE_ENC), -F8MAX, F8MAX).astype(f8)
        encB = encT.astype(bf)
        # em = exact linear term - quantized linear term + mask
        elin = enc[lo:hi] @ u                               # [BPC, S]
        elin8 = np.einsum("e,bes->bs", u8, encH.astype(np.float32),
                          optimize=True)
        em = (elin - np.float32(UNSCALE) * elin8 + maskadd[lo:hi]).astype(bf)

        dc = decf[lo:hi]                                    # [BPC, H]
        decf_arr = np.ascontiguousarray(
            dc.reshape(BPC, KT, P).transpose(2, 1, 0).reshape(P, KT * BPC)
        ).astype(np.float32)

        in_maps.append({
            "encH": encH,
            "encB": encB,
            "wsT": W8,
            "decf": decf_arr,
            "vv": vv,
            "emask": em,
        })
    return in_maps


def kernel(decoder_hidden, encoder_outputs, src_mask, W_h, W_s, v, _trace=False):
    if "nc" not in _CACHE:
        _CACHE["nc"] = _build()
    nc = _CACHE["nc"]

    in_maps = _prep_inputs(
        np.asarray(decoder_hidden, dtype=np.float32),
        np.asarray(encoder_outputs, dtype=np.float32),
        np.asarray(src_mask),
        np.asarray(W_h, dtype=np.float32),
        np.asarray(W_s, dtype=np.float32),
        np.asarray(v, dtype=np.float32),
    )

    res = run_bass_kernel_spmd(nc, in_maps, core_ids=list(range(NC)), trace=_trace)
    _CACHE["last_result"] = res

    context = np.empty((B, E), dtype=np.float32)
    attn = np.empty((B, S), dtype=np.float32)
    for c in range(NC):
        lo, hi = c * BPC, (c + 1) * BPC
        attn[lo:hi] = res.results[c]["attn"]
        raw = res.results[c]["ctxr"]  # [P, BPC*ET]
        context[lo:hi] = raw.reshape(P, BPC, ET).transpose(1, 2, 0).reshape(BPC, E)
    return context, attn


# revision 8
# speedup vs baseline: 1.2447x; 1.2447x over previous
"""Bahdanau additive attention on 8 Trainium2 NeuronCores — fp8 DoubleRow.

Math (per batch b):
    dec_f  = decoder_hidden @ W_h                     [H]   (host, fp32)
    enc_f  = encoder_outputs[b] @ W_s                 [S, H]
    energy = tanh(dec_f + enc_f) @ v + addmask        [S]
    attn   = softmax(energy)                          [S]
    context= attn @ encoder_outputs[b]                [2H]

Sharding: data-parallel over batch, 8 batches per core, weights replicated.

The big matmul runs in fp8e4 with perf_mode=DoubleRow (2 contraction
tiles per matmul, 2x PE throughput): host pre-scales enc by 2^4 and W_s
by 2^8 (clip +-240), quantizes to e4m3; the tanh activation applies
scale=2^-12 with dec_f (host fp32) as per-partition bias.

fp8 error rescue (zero device cost): the host folds the exact linear
correction into the mask row:
    em = enc@(W_s@v) - 2^-12 * enc8@(W8@v) + mask
so device energy = em + v.tanh(x8) = exact_linear + v.(tanh(x8) - x8),
i.e. the quantization error only enters through d/dx[x - tanh(x)] =
tanh^2(x) (mean ~0.24) instead of tanh'(x) (mean ~0.44). dec_f cancels
identically so it never appears in em.

Engine split. The PE p-state drops on every stall (cold matmuls run 2x
slower), so the PE stream is kept to an uninterrupted run of DoubleRow
mains; per batch it additionally executes only the 4 cross-partition
v-sum matmuls + 4 K=1 em-row adds, emitted AFTER the next batch's k=0
mains so their DVE input (the v-chain) is always ready. Everything else
lives on other engines: the v-contraction is a DVE
scalar_tensor_tensor chain (acc_k = hid_k*v_k + acc_{k-1}); softmax
total/broadcast run on GpSimd (partition_all_reduce /
partition_broadcast) so the PE never touches them; context is one
fused scalar_tensor_tensor (product + free-dim accumulate) per e-tile
against an UNSCALED bf16 copy of encT, spread 4-per-k-iteration across
the next batch so the DVE queue never backs up. The last batch's
context runs on the (then idle) PE from an fp8 natural-layout copy:
unnormalized bf16 exp weights are broadcast via a DRAM scratch row,
DMA-transposed into matmul columns, and 1/sum is applied by the scalar
engine during the PSUM->SBUF copy (the fp8 1/16 scale is folded on the
host).
"""

import numpy as np
import ml_dtypes

import concourse.bacc as bacc
import concourse.mybir as mybir
import concourse.bass_isa as bass_isa
import concourse.tile as tile
from concourse.bass_utils import run_bass_kernel_spmd

# Problem shapes (hardcoded per contest rules).
B, S, H = 64, 2048, 1024
E = 2 * H            # encoder feature dim
NC = 8               # cores
BPC = B // NC        # batches per core
P = 128              # partitions
ET = E // P          # 16 e-tiles (contraction tiles of main matmul)
EPAIR = ET // 2      # 8 e-tile pairs (DoubleRow processes 2 at once)
KT = H // P          # 8 k-tiles (hidden dim tiles)
SC = S // 512        # 4 s-chunks of 512
NEG_BIG = -1e10

SCALE_ENC = 16.0     # enc pre-scale before e4m3 cast
SCALE_W = 256.0      # W_s pre-scale before e4m3 cast
UNSCALE = 1.0 / (SCALE_ENC * SCALE_W)
F8MAX = 240.0        # TRN e4m3 max normal

F32 = mybir.dt.float32
BF16 = mybir.dt.bfloat16
F8 = mybir.dt.float8e4

_CACHE = {}


def _build():
    nc = bacc.Bacc("TRN2", target_bir_lowering=False, debug=False, num_devices=NC)

    encH_d = nc.dram_tensor("encH", [BPC, E, S], F8, kind="ExternalInput")
    encB_d = nc.dram_tensor("encB", [BPC, E, S], BF16, kind="ExternalInput")
    ws_d = nc.dram_tensor("wsT", [E, H], F8, kind="ExternalInput")
    decf_d = nc.dram_tensor("decf", [P, KT * BPC], F32, kind="ExternalInput")
    v_d = nc.dram_tensor("vv", [P, KT], BF16, kind="ExternalInput")
    em_d = nc.dram_tensor("emask", [BPC, S], BF16, kind="ExternalInput")
    # scaled fp8 natural-layout copy of the LAST local batch (tail fast path)
    encN_d = nc.dram_tensor("encN", [S, E], F8, kind="ExternalInput")

    attn_d = nc.dram_tensor("attn", [BPC, S], F32, kind="ExternalOutput")
    ctx_d = nc.dram_tensor("ctxr", [P, BPC * ET], F32, kind="ExternalOutput")
    ctxl_d = nc.dram_tensor("ctxl", [1, E], F32, kind="ExternalOutput")

    with tile.TileContext(nc) as tc:
        with (
            tc.tile_pool(name="const", bufs=1) as cpool,
            tc.tile_pool(name="psum_mm", bufs=7, space="PSUM") as mmp,
            tc.tile_pool(name="psum_en", bufs=1, space="PSUM") as enp,
        ):
            # ---- persistent constants ----
            v_sb = cpool.tile([P, KT], BF16)
            v_f32 = cpool.tile([P, KT], F32)
            ws_sb = cpool.tile([P, ET, H], F8)
            decf_sb = cpool.tile([P, KT, BPC], F32)
            one_one = cpool.tile([1, 1], BF16)
            nc.vector.memset(one_one[:], 1.0)
            ones_col = cpool.tile([P, 1], BF16)
            nc.vector.memset(ones_col[:], 1.0)
            # energy lives packed on partitions {0,32,64,96} of ONE psum
            # bank; ssum_t holds the per-chunk exp partial sums (its other
            # partitions stay zero so a partition all-reduce gives the total)
            ssum_t = cpool.tile([97, 1], F32)
            nc.vector.memset(ssum_t[:], 0.0)
            ssum_red = cpool.tile([97, 1], F32)
            ctx_acc = cpool.tile([P, BPC * ET], F32)

            # ---- batch pipeline ----
            with (
                tc.tile_pool(name="encp", bufs=8) as encp,
                tc.tile_pool(name="encbp", bufs=5) as encbp,
                tc.tile_pool(name="work", bufs=2) as wkp,
                tc.tile_pool(name="dscr", bufs=1, space="DRAM") as dscr,
            ):
                # startup: tiny tensors, then just the k=0 columns of W_s so
                # batch 0's first mains aren't gated on the full weight load.
                nc.sync.dma_start(v_sb[:], v_d.ap())
                nc.vector.tensor_copy(v_f32[:], v_sb[:])
                nc.sync.dma_start(decf_sb[:], decf_d.ap())
                ws_ap = ws_d.ap().rearrange("(t p) k -> p t k", p=P)
                nc.sync.dma_start(ws_sb[:, :, 0:P], ws_ap[:, :, 0:P])

                state = {}

                def emit_load(b):
                    quarters = []
                    for q in range(4):
                        encq = encp.tile([P, 4, S], F8, tag="encq", name=f"encq_{b}_{q}")
                        nc.sync.dma_start(
                            encq[:],
                            encH_d.ap()[b, q * 512:(q + 1) * 512, :].rearrange(
                                "(t p) s -> p t s", p=P
                            ),
                        )
                        quarters.append(encq)
                    mask_st = wkp.tile([1, S], BF16, tag="mask", bufs=2, name=f"mask_{b}")
                    nc.sync.dma_start(mask_st[:], em_d.ap()[b:b + 1, :])
                    state[b] = (quarters, mask_st)

                def emit_loadB(b):
                    quarters = []
                    for q in range(4):
                        encq = encbp.tile([P, 4, S], BF16, tag="encb", name=f"encb_{b}_{q}")
                        nc.sync.dma_start(
                            encq[:],
                            encB_d.ap()[b, q * 512:(q + 1) * 512, :].rearrange(
                                "(t p) s -> p t s", p=P
                            ),
                        )
                        quarters.append(encq)
                    state[b, "ctxq"] = quarters

                def emit_energy_tail(b):
                    """Collapse batch b's v-chain results into the packed
                    energy PSUM rows (cross-partition ones-matmul) and add
                    the em row (linear correction + mask). Emitted after
                    batch b+1's k=0 mains so the chain is always ready by
                    the time the PE gets here (no stall, no p-state drop)."""
                    energy_ps = state[b, "energy"]
                    _, mask_st = state[b]
                    for sc in range(SC):
                        nc.tensor.matmul(
                            energy_ps[32 * sc:32 * sc + 1, :],
                            ones_col[:],
                            state[b, "vacc", sc][:],
                            start=True,
                            stop=False,
                            skip_group_check=True,
                            tile_position=(0, 32 * sc),
                        )
                        nc.tensor.matmul(
                            energy_ps[32 * sc:32 * sc + 1, :],
                            one_one[:],
                            mask_st[0:1, sc * 512:(sc + 1) * 512],
                            start=False,
                            stop=True,
                            skip_group_check=True,
                            tile_position=(0, 32 * sc),
                        )

                def emit_exp(b):
                    """Exp over batch b's energy PSUM (4 packed chunk rows)."""
                    energy_ps = state[b, "energy"]
                    exps = wkp.tile([97, 512], F32, tag="exps", bufs=1,
                                    name=f"exps_{b}")
                    for c in range(SC):
                        nc.scalar.activation(
                            exps[32 * c:32 * c + 1, :],
                            energy_ps[32 * c:32 * c + 1, :],
                            mybir.ActivationFunctionType.Exp,
                            accum_out=ssum_t[32 * c:32 * c + 1, :],
                        )
                    state[b, "exp"] = exps

                def emit_sum(b):
                    """Exp total (GpSimd all-reduce), reciprocal (DVE) and
                    1/sum broadcast to 128 partitions (GpSimd) — no PE."""
                    nc.gpsimd.partition_all_reduce(ssum_red[:], ssum_t[:], 97,
                                                   bass_isa.ReduceOp.add)
                    sinv = wkp.tile([1, 1], F32, tag="sinv", name=f"sinv_{b}")
                    nc.vector.reciprocal(sinv[:], ssum_red[0:1, :])
                    sinv_all = wkp.tile([P, 1], F32, tag="sinv_all",
                                        name=f"sinv_all_{b}")
                    nc.gpsimd.partition_broadcast(sinv_all[:], sinv[:])
                    state[b, "sinv_all"] = sinv_all

                def emit_post_head(b):
                    """Normalized attn row -> DRAM output + SWDGE broadcast
                    of the row to 128 partitions for the context multiply."""
                    exps = state[b, "exp"]
                    sinv_all = state[b, "sinv_all"]
                    attn_row = wkp.tile([97, 512], F32, tag="attn_row", bufs=1,
                                        name=f"attn_row_{b}")
                    nc.vector.tensor_scalar_mul(attn_row[:], exps[:],
                                                sinv_all[0:97, :])
                    for c in range(SC):
                        nc.sync.dma_start(
                            attn_d.ap()[b:b + 1, c * 512:(c + 1) * 512],
                            attn_row[32 * c:32 * c + 1, :],
                        )
                    attn_bc = wkp.tile([P, S], BF16, tag="attn_bc", bufs=1,
                                       name=f"attn_bc_{b}")
                    nc.gpsimd.dma_start(
                        attn_bc[:], attn_d.ap()[b:b + 1, :].broadcast_to((P, S))
                    )
                    state[b, "attn_bc"] = attn_bc

                def emit_ctx(b, elist):
                    """Context for e-tiles in elist: one fused DVE op each."""
                    ctxq = state[b, "ctxq"]
                    attn_bc = state[b, "attn_bc"]
                    for e in elist:
                        q, qt = divmod(e, 4)
                        scr = wkp.tile([P, S], BF16, tag="scr", bufs=1,
                                       name=f"scr_{b}_{e}")
                        acc = ctx_acc[:, b * ET + e:b * ET + e + 1]
                        nc.vector.scalar_tensor_tensor(
                            scr[:],
                            ctxq[q][:, qt, :],
                            1.0,
                            attn_bc[:],
                            mybir.AluOpType.mult,
                            mybir.AluOpType.mult,
                            accum_out=acc,
                        )

                def emit_post_last(b):
                    """Tail fast path for the final batch: broadcast
                    UNNORMALIZED bf16 exps via a DRAM scratch row, context on
                    the (now idle) PE from the fp8 natural-layout copy with a
                    DMA xbar transpose for the attn columns; 1/sum applied by
                    the scalar engine during the PSUM->SBUF copy (the fp8
                    1/16 enc scale is folded on the host)."""
                    exps = state[b, "exp"]
                    sinv_all = state[b, "sinv_all"]
                    exps_bf = wkp.tile([97, 512], BF16, tag="exps_bf", bufs=1,
                                       name=f"exps_bf_{b}")
                    nc.vector.tensor_copy(exps_bf[:], exps[:])
                    bsc = dscr.tile([1, S], BF16, name=f"bsc_{b}")
                    for c in range(SC):
                        nc.sync.dma_start(
                            bsc[0:1, c * 512:(c + 1) * 512],
                            exps_bf[32 * c:32 * c + 1, :],
                        )
                    attn_row = wkp.tile([97, 512], F32, tag="attn_row", bufs=1,
                                        name=f"attn_row_{b}")
                    nc.vector.tensor_scalar_mul(attn_row[:], exps[:],
                                                sinv_all[0:97, :])
                    for c in range(SC):
                        nc.sync.dma_start(
                            attn_d.ap()[b:b + 1, c * 512:(c + 1) * 512],
                            attn_row[32 * c:32 * c + 1, :],
                        )
                    attnT = wkp.tile([P, ET], BF16, tag="attnT", bufs=1,
                                     name=f"attnT_{b}")
                    nc.sync.dma_start_transpose(
                        attnT[:],
                        bsc[0:1, :].rearrange("o (t p) -> (o t) p", p=P),
                    )
                    natq = state["natq"]
                    ctxps = enp.tile([97, 512], F32, tag="energy",
                                     name="ctxps_last")
                    for t in range(ET):
                        for c in range(SC):
                            nc.tensor.matmul(
                                ctxps[32 * c:32 * c + 1, :],
                                attnT[:, t:t + 1],
                                natq[t // 4][:, t % 4, c * 512:(c + 1) * 512],
                                start=(t == 0),
                                stop=(t == ET - 1),
                                skip_group_check=True,
                                tile_position=(0, 32 * c),
                            )
                    ctx_row = wkp.tile([97, 512], F32, tag="ctx_row", bufs=1,
                                       name="ctx_row_last")
                    for c in range(SC):
                        nc.scalar.activation(
                            ctx_row[32 * c:32 * c + 1, :],
                            ctxps[32 * c:32 * c + 1, :],
                            mybir.ActivationFunctionType.Copy,
                            scale=sinv_all[32 * c:32 * c + 1, :],
                        )
                        nc.sync.dma_start(
                            ctxl_d.ap()[0:1, c * 512:(c + 1) * 512],
                            ctx_row[32 * c:32 * c + 1, :],
                        )

                # batch 0's data next on the DMA queue, then the rest of W_s
                emit_load(0)
                nc.sync.dma_start(ws_sb[:, :, P:H], ws_ap[:, :, P:H])
                emit_load(1)
                for b in range(BPC):
                    quarters, mask_st = state[b]
                    energy_ps = enp.tile([97, 512], F32, tag="energy", name=f"energy_{b}")
                    state[b, "energy"] = energy_ps

                    hid_prev = None
                    for k in range(KT):
                        if k == 1 and b > 0:
                            emit_energy_tail(b - 1)
                            emit_exp(b - 1)
                        if k == 2 and b > 0:
                            emit_sum(b - 1)
                        if k == 3 and b > 0:
                            emit_post_head(b - 1)
                        if k == 3 and b < BPC - 1:
                            emit_loadB(b)
                        if k == 4 and b + 2 < BPC:
                            emit_load(b + 2)
                        if k == 4 and b == BPC - 1:
                            natq = []
                            for q in range(4):
                                nq = encp.tile([P, 4, E], F8, tag="encq",
                                               name=f"natq_{q}")
                                nc.sync.dma_start(
                                    nq[:],
                                    encN_d.ap()[q * 512:(q + 1) * 512, :].rearrange(
                                        "(t p) e -> p t e", p=P
                                    ),
                                )
                                natq.append(nq)
                            state["natq"] = natq
                        pss = []
                        for sc in range(SC):
                            ps = mmp.tile([P, 512], F32, tag="mm",
                                          name=f"ps_{b}_{k}_{sc}")
                            pss.append(ps)
                        for ep in range(EPAIR):
                            q, qt = divmod(2 * ep, 4)
                            for sc in range(SC):
                                nc.tensor.matmul(
                                    pss[sc][:],
                                    ws_sb[:, 2 * ep:2 * ep + 2, k * P:(k + 1) * P],
                                    quarters[q][:, qt:qt + 2, sc * 512:(sc + 1) * 512],
                                    start=(ep == 0),
                                    stop=(ep == EPAIR - 1),
                                    perf_mode=mybir.MatmulPerfMode.DoubleRow,
                                )
                        # v-chain steps for k-1's hids (DVE)
                        if hid_prev is not None:
                            for sc in range(SC):
                                vacc = wkp.tile([P, 512], BF16, tag="vacc",
                                                bufs=8, name=f"vacc_{b}_{k-1}_{sc}")
                                if k == 1:
                                    nc.vector.tensor_scalar_mul(
                                        vacc[:], hid_prev[sc][:],
                                        v_f32[:, k - 1:k])
                                else:
                                    nc.vector.scalar_tensor_tensor(
                                        vacc[:], hid_prev[sc][:],
                                        v_f32[:, k - 1:k],
                                        state[b, "vacc", sc][:],
                                        mybir.AluOpType.mult,
                                        mybir.AluOpType.add,
                                    )
                                state[b, "vacc", sc] = vacc
                        # context for the previous batch: 4 e-tiles per k
                        if b > 0 and 3 <= k <= 6:
                            emit_ctx(b - 1, range(4 * (k - 3), 4 * (k - 2)))
                        hids = []
                        for sc in range(SC):
                            hid = wkp.tile([P, 512], BF16, tag="hid", bufs=8,
                                           name=f"hid_{b}_{k}_{sc}")
                            nc.scalar.activation(
                                hid[:],
                                pss[sc][:],
                                mybir.ActivationFunctionType.Tanh,
                                bias=decf_sb[:, k, b:b + 1],
                                scale=UNSCALE,
                            )
                            hids.append(hid)
                        hid_prev = hids
                    # final v-chain steps (k = KT-1)
                    for sc in range(SC):
                        vacc = wkp.tile([P, 512], BF16, tag="vacc", bufs=8,
                                        name=f"vacc_{b}_{KT-1}_{sc}")
                        nc.vector.scalar_tensor_tensor(
                            vacc[:], hid_prev[sc][:], v_f32[:, KT - 1:KT],
                            state[b, "vacc", sc][:],
                            mybir.AluOpType.mult, mybir.AluOpType.add,
                        )
                        state[b, "vacc", sc] = vacc
                emit_energy_tail(BPC - 1)
                emit_exp(BPC - 1)
                emit_sum(BPC - 1)
                emit_post_last(BPC - 1)

            nc.sync.dma_start(ctx_d.ap()[:], ctx_acc[:])

    nc.compile()
    return nc


def _prep_inputs(decoder_hidden, encoder_outputs, src_mask, W_h, W_s, v):
    bf = ml_dtypes.bfloat16
    f8 = ml_dtypes.float8_e4m3

    enc = np.asarray(encoder_outputs, np.float32)          # [B, S, E]
    W_s = np.asarray(W_s, np.float32)
    v_b = np.asarray(v, np.float32).astype(bf).astype(np.float32)

    # quantized weights + linear-correction vectors
    W8 = np.clip(W_s * SCALE_W, -F8MAX, F8MAX).astype(f8)
    u = W_s @ v_b                                          # [E] exact
    u8 = W8.astype(np.float32) @ v_b                       # [E] of quantized W

    decf = np.asarray(decoder_hidden, np.float32) @ np.asarray(W_h, np.float32)

    vv = np.ascontiguousarray(v_b.astype(bf).reshape(KT, P).T)
    maskadd = np.where(np.asarray(src_mask) == 0, np.float32(NEG_BIG),
                       np.float32(0.0))

    in_maps = []
    for c in range(NC):
        lo, hi = c * BPC, (c + 1) * BPC
        encT = np.ascontiguousarray(enc[lo:hi].transpose(0, 2, 1))  # [BPC,E,S]
        encH = np.clip(encT * np.float32(SCALE_ENC), -F8MAX, F8MAX).astype(f8)
        encB = encT.astype(bf)
        # em = exact linear term - quantized linear term + mask
        elin = enc[lo:hi] @ u                               # [BPC, S]
        elin8 = np.einsum("e,bes->bs", u8, encH.astype(np.float32),
                          optimize=True)
        em = (elin - np.float32(UNSCALE) * elin8 + maskadd[lo:hi]).astype(bf)

        dc = decf[lo:hi]                                    # [BPC, H]
        decf_arr = np.ascontiguousarray(
            dc.reshape(BPC, KT, P).transpose(2, 1, 0).reshape(P, KT * BPC)
        ).astype(np.float32)

        encN = np.clip(enc[hi - 1] * np.float32(SCALE_ENC), -F8MAX,
                       F8MAX).astype(f8)                    # [S, E] fp8

        in_maps.append({
            "encH": encH,
            "encB": encB,
            "wsT": W8,
            "decf": decf_arr,
            "vv": vv,
            "emask": em,
            "encN": encN,
        })
    return in_maps


def kernel(decoder_hidden, encoder_outputs, src_mask, W_h, W_s, v, _trace=False):
    if "nc" not in _CACHE:
        _CACHE["nc"] = _build()
    nc = _CACHE["nc"]

    in_maps = _prep_inputs(
        np.asarray(decoder_hidden, dtype=np.float32),
        np.asarray(encoder_outputs, dtype=np.float32),
        np.asarray(src_mask),
        np.asarray(W_h, dtype=np.float32),
        np.asarray(W_s, dtype=np.float32),
        np.asarray(v, dtype=np.float32),
    )

    res = run_bass_kernel_spmd(nc, in_maps, core_ids=list(range(NC)), trace=_trace)
    _CACHE["last_result"] = res

    context = np.empty((B, E), dtype=np.float32)
    attn = np.empty((B, S), dtype=np.float32)
    for c in range(NC):
        lo, hi = c * BPC, (c + 1) * BPC
        attn[lo:hi] = res.results[c]["attn"]
        raw = res.results[c]["ctxr"]  # [P, BPC*ET]
        context[lo:hi] = raw.reshape(P, BPC, ET).transpose(1, 2, 0).reshape(BPC, E)
        # last local batch: PE tail fast path, fp8 enc scale folded out here
        context[hi - 1] = res.results[c]["ctxl"][0] * np.float32(1.0 / SCALE_ENC)
    return context, attn


# revision 16
# speedup vs baseline: 1.3083x; 1.0511x over previous
"""Bahdanau additive attention on 8 Trainium2 NeuronCores — fp8 DoubleRow.

Math (per batch b):
    dec_f  = decoder_hidden @ W_h                     [H]   (host, fp32)
    enc_f  = encoder_outputs[b] @ W_s                 [S, H]
    energy = tanh(dec_f + enc_f) @ v + addmask        [S]
    attn   = softmax(energy)                          [S]
    context= attn @ encoder_outputs[b]                [2H]

Sharding: data-parallel over batch, 8 batches per core, weights replicated.

The big matmul runs in fp8e4 with perf_mode=DoubleRow (2 contraction
tiles per matmul, 2x PE throughput): host pre-scales enc by 2^4 and W_s
by 2^8 (clip +-240), quantizes to e4m3; the tanh activation applies
scale=2^-12 with dec_f (host fp32) as per-partition bias.

fp8 error rescue (zero device cost): the host folds the exact linear
correction into the mask row:
    em = enc@(W_s@v) - 2^-12 * enc8@(W8@v) + mask
so device energy = em + v.tanh(x8) = exact_linear + v.(tanh(x8) - x8),
i.e. the quantization error only enters through d/dx[x - tanh(x)] =
tanh^2(x) (mean ~0.24) instead of tanh'(x) (mean ~0.44). dec_f cancels
identically so it never appears in em.

Engine split. The PE p-state drops on every stall (cold matmuls run 2x
slower), so the PE stream is kept to an uninterrupted run of DoubleRow
mains; per batch it additionally executes only the 4 cross-partition
v-sum matmuls + 4 K=1 em-row adds, emitted AFTER the next batch's k=0
mains so their DVE input (the v-chain) is always ready. Everything else
lives on other engines: the v-contraction is a DVE
scalar_tensor_tensor chain (acc_k = hid_k*v_k + acc_{k-1}); softmax
total/broadcast run on GpSimd (partition_all_reduce /
partition_broadcast) so the PE never touches them; context is one
fused scalar_tensor_tensor (product + free-dim accumulate) per e-tile
against an UNSCALED bf16 copy of encT, spread 4-per-k-iteration across
the next batch so the DVE queue never backs up. The last batch's
context runs on the (then idle) PE from an fp8 natural-layout copy:
unnormalized bf16 exp weights are broadcast via a DRAM scratch row,
DMA-transposed into matmul columns, and 1/sum is applied by the scalar
engine during the PSUM->SBUF copy (the fp8 1/16 scale is folded on the
host).
"""

import numpy as np
import ml_dtypes

import concourse.bacc as bacc
import concourse.mybir as mybir
import concourse.bass_isa as bass_isa
import concourse.tile as tile
from concourse.bass_utils import run_bass_kernel_spmd

# Problem shapes (hardcoded per contest rules).
B, S, H = 64, 2048, 1024
E = 2 * H            # encoder feature dim
NC = 8               # cores
BPC = B // NC        # batches per core
P = 128              # partitions
ET = E // P          # 16 e-tiles (contraction tiles of main matmul)
EPAIR = ET // 2      # 8 e-tile pairs (DoubleRow processes 2 at once)
KT = H // P          # 8 k-tiles (hidden dim tiles)
SC = S // 512        # 4 s-chunks of 512
NEG_BIG = -1e10

SCALE_ENC = 16.0     # enc pre-scale before e4m3 cast
SCALE_W = 256.0      # W_s pre-scale before e4m3 cast
UNSCALE = 1.0 / (SCALE_ENC * SCALE_W)
F8MAX = 240.0        # TRN e4m3 max normal

F32 = mybir.dt.float32
BF16 = mybir.dt.bfloat16
F8 = mybir.dt.float8e4

_CACHE = {}


def _build():
    nc = bacc.Bacc("TRN2", target_bir_lowering=False, debug=False, num_devices=NC)

    encH_d = nc.dram_tensor("encH", [BPC, 4, P, 4 * S], F8, kind="ExternalInput")
    encB_d = nc.dram_tensor("encB", [BPC, 4, P, 4 * S], BF16, kind="ExternalInput")
    ws_d = nc.dram_tensor("wsT", [P, KT * ET * P], F8, kind="ExternalInput")
    decf_d = nc.dram_tensor("decf", [P, KT * BPC], F32, kind="ExternalInput")
    v_d = nc.dram_tensor("vv", [P, KT], BF16, kind="ExternalInput")
    em_d = nc.dram_tensor("emask", [BPC, S], BF16, kind="ExternalInput")
    # scaled fp8 natural-layout copy of the LAST local batch (tail fast path)
    encN_d = nc.dram_tensor("encN", [4, P, 4 * E], F8, kind="ExternalInput")

    attn_d = nc.dram_tensor("attn", [BPC, S], F32, kind="ExternalOutput")
    ctx_d = nc.dram_tensor("ctxr", [P, BPC * ET], F32, kind="ExternalOutput")
    ctxl_d = nc.dram_tensor("ctxl", [1, E], F32, kind="ExternalOutput")

    with tile.TileContext(nc) as tc:
        with (
            tc.tile_pool(name="const", bufs=1) as cpool,
            tc.tile_pool(name="psum_mm", bufs=8, space="PSUM") as mmp,
        ):
            # ---- persistent constants ----
            v_sb = cpool.tile([P, KT], BF16)
            v_f32 = cpool.tile([P, KT], F32)
            ws_sb = cpool.tile([P, KT, ET, P], F8)
            decf_sb = cpool.tile([P, KT, BPC], F32)
            one_one = cpool.tile([1, 1], BF16)
            nc.vector.memset(one_one[:], 1.0)
            ones_col = cpool.tile([P, 1], BF16)
            nc.vector.memset(ones_col[:], 1.0)
            # energy lives packed on partitions {0,32,64,96} of ONE psum
            # bank; ssum_t holds the per-chunk exp partial sums (its other
            # partitions stay zero so a partition all-reduce gives the total)
            ssum_t = cpool.tile([97, 1], F32)
            nc.vector.memset(ssum_t[:], 0.0)
            ssum_red = cpool.tile([97, 1], F32)
            ctx_acc = cpool.tile([P, BPC * ET], F32)

            # ---- batch pipeline ----
            with (
                tc.tile_pool(name="encp", bufs=8) as encp,
                tc.tile_pool(name="encbp", bufs=5) as encbp,
                tc.tile_pool(name="work", bufs=2) as wkp,
                tc.tile_pool(name="dscr", bufs=1, space="DRAM") as dscr,
            ):
                # startup: tiny tensors, then just the k=0 columns of W_s so
                # batch 0's first mains aren't gated on the full weight load.
                nc.sync.dma_start(v_sb[:], v_d.ap())
                nc.vector.tensor_copy(v_f32[:], v_sb[:])
                nc.sync.dma_start(decf_sb[:], decf_d.ap())
                ws_ap = ws_d.ap().rearrange("p (k t c) -> p k t c", k=KT, t=ET)
                nc.sync.dma_start(ws_sb[:, 0:1, :, :], ws_ap[:, 0:1, :, :])

                state = {}

                def emit_load(b, split=False):
                    quarters = []
                    for q in range(4):
                        encq = encp.tile([P, 4, S], F8, tag="encq", name=f"encq_{b}_{q}")
                        # split=True: alternate issue queues (SP/ACT) so the
                        # startup-critical batch-0 quarters load in parallel
                        eng = nc.scalar if (split and q % 2) else nc.sync
                        eng.dma_start(
                            encq[:],
                            encH_d.ap()[b, q].rearrange("p (t s) -> p t s", t=4),
                        )
                        quarters.append(encq)
                    mask_st = wkp.tile([1, S], BF16, tag="mask", bufs=2, name=f"mask_{b}")
                    nc.sync.dma_start(mask_st[:], em_d.ap()[b:b + 1, :])
                    state[b] = (quarters, mask_st)

                def emit_loadB(b):
                    # issued on the ACT hwdge queue: a second DMA rail so the
                    # context copy never starves the PE-critical encH stream
                    quarters = []
                    for q in range(4):
                        encq = encbp.tile([P, 4, S], BF16, tag="encb", name=f"encb_{b}_{q}")
                        nc.scalar.dma_start(
                            encq[:],
                            encB_d.ap()[b, q].rearrange("p (t s) -> p t s", t=4),
                        )
                        quarters.append(encq)
                    state[b, "ctxq"] = quarters

                def emit_energy_tail(b):
                    """Collapse batch b's v-chain results into the packed
                    energy PSUM rows (cross-partition ones-matmul) and add
                    the em row (linear correction + mask). Emitted after
                    batch b+1's k=0 mains so the chain is always ready by
                    the time the PE gets here (no stall, no p-state drop)."""
                    energy_ps = mmp.tile([97, 512], F32, tag="mm",
                                         name=f"energy_{b}")
                    state[b, "energy"] = energy_ps
                    _, mask_st = state[b]
                    for sc in range(SC):
                        nc.tensor.matmul(
                            energy_ps[32 * sc:32 * sc + 1, :],
                            ones_col[:],
                            state[b, "vacc", sc][:],
                            start=True,
                            stop=False,
                            skip_group_check=True,
                            tile_position=(0, 32 * sc),
                        )
                        nc.tensor.matmul(
                            energy_ps[32 * sc:32 * sc + 1, :],
                            one_one[:],
                            mask_st[0:1, sc * 512:(sc + 1) * 512],
                            start=False,
                            stop=True,
                            skip_group_check=True,
                            tile_position=(0, 32 * sc),
                        )

                def emit_exp(b):
                    """Exp over batch b's energy PSUM (4 packed chunk rows)."""
                    energy_ps = state[b, "energy"]
                    exps = wkp.tile([97, 512], F32, tag="exps", bufs=1,
                                    name=f"exps_{b}")
                    for c in range(SC):
                        nc.scalar.activation(
                            exps[32 * c:32 * c + 1, :],
                            energy_ps[32 * c:32 * c + 1, :],
                            mybir.ActivationFunctionType.Exp,
                            accum_out=ssum_t[32 * c:32 * c + 1, :],
                        )
                    state[b, "exp"] = exps

                def emit_sum(b):
                    """Exp total (GpSimd all-reduce), reciprocal (DVE) and
                    1/sum broadcast to 128 partitions (GpSimd) — no PE."""
                    nc.gpsimd.partition_all_reduce(ssum_red[:], ssum_t[:], 97,
                                                   bass_isa.ReduceOp.add)
                    sinv = wkp.tile([1, 1], F32, tag="sinv", name=f"sinv_{b}")
                    nc.vector.reciprocal(sinv[:], ssum_red[0:1, :])
                    sinv_all = wkp.tile([P, 1], F32, tag="sinv_all",
                                        name=f"sinv_all_{b}")
                    nc.gpsimd.partition_broadcast(sinv_all[:], sinv[:])
                    state[b, "sinv_all"] = sinv_all

                def emit_post_head(b):
                    """Normalized attn row -> DRAM output + SWDGE broadcast
                    of the row to 128 partitions for the context multiply."""
                    exps = state[b, "exp"]
                    sinv_all = state[b, "sinv_all"]
                    attn_row = wkp.tile([97, 512], F32, tag="attn_row", bufs=1,
                                        name=f"attn_row_{b}")
                    nc.vector.tensor_scalar_mul(attn_row[:], exps[:],
                                                sinv_all[0:97, :])
                    for c in range(SC):
                        nc.sync.dma_start(
                            attn_d.ap()[b:b + 1, c * 512:(c + 1) * 512],
                            attn_row[32 * c:32 * c + 1, :],
                        )
                    attn_bc = wkp.tile([P, S], BF16, tag="attn_bc", bufs=1,
                                       name=f"attn_bc_{b}")
                    nc.gpsimd.dma_start(
                        attn_bc[:], attn_d.ap()[b:b + 1, :].broadcast_to((P, S))
                    )
                    state[b, "attn_bc"] = attn_bc

                def emit_ctx(b, elist):
                    """Context for e-tiles in elist: one fused DVE op each."""
                    ctxq = state[b, "ctxq"]
                    attn_bc = state[b, "attn_bc"]
                    for e in elist:
                        q, qt = divmod(e, 4)
                        scr = wkp.tile([P, S], BF16, tag="scr", bufs=1,
                                       name=f"scr_{b}_{e}")
                        acc = ctx_acc[:, b * ET + e:b * ET + e + 1]
                        nc.vector.scalar_tensor_tensor(
                            scr[:],
                            ctxq[q][:, qt, :],
                            1.0,
                            attn_bc[:],
                            mybir.AluOpType.mult,
                            mybir.AluOpType.mult,
                            accum_out=acc,
                        )
                    if elist[-1] == ET - 1:
                        nc.sync.dma_start(
                            ctx_d.ap()[:, b * ET:(b + 1) * ET],
                            ctx_acc[:, b * ET:(b + 1) * ET],
                        )

                def emit_post_last(b):
                    """Tail fast path for the final batch: broadcast
                    UNNORMALIZED bf16 exps via a DRAM scratch row, context on
                    the (now idle) PE from the fp8 natural-layout copy with a
                    DMA xbar transpose for the attn columns; 1/sum applied by
                    the scalar engine during the PSUM->SBUF copy (the fp8
                    1/16 enc scale is folded on the host)."""
                    exps = state[b, "exp"]
                    sinv_all = state[b, "sinv_all"]
                    exps_bf = wkp.tile([97, 512], BF16, tag="exps_bf", bufs=1,
                                       name=f"exps_bf_{b}")
                    bsc = dscr.tile([1, S], BF16, name=f"bsc_{b}")
                    for c in range(SC):
                        nc.vector.tensor_copy(exps_bf[32 * c:32 * c + 1, :],
                                              exps[32 * c:32 * c + 1, :])
                        nc.sync.dma_start(
                            bsc[0:1, c * 512:(c + 1) * 512],
                            exps_bf[32 * c:32 * c + 1, :],
                        )
                    attn_row = wkp.tile([97, 512], F32, tag="attn_row", bufs=1,
                                        name=f"attn_row_{b}")
                    nc.vector.tensor_scalar_mul(attn_row[:], exps[:],
                                                sinv_all[0:97, :])
                    for c in range(SC):
                        nc.sync.dma_start(
                            attn_d.ap()[b:b + 1, c * 512:(c + 1) * 512],
                            attn_row[32 * c:32 * c + 1, :],
                        )
                    attnT = wkp.tile([P, ET], BF16, tag="attnT", bufs=1,
                                     name=f"attnT_{b}")
                    nc.sync.dma_start_transpose(
                        attnT[:],
                        bsc[0:1, :].rearrange("o (t p) -> (o t) p", p=P),
                    )
                    natq = state["natq"]
                    ctxps = mmp.tile([97, 512], F32, tag="mm",
                                     name="ctxps_last")
                    for t in range(ET):
                        for c in range(SC):
                            nc.tensor.matmul(
                                ctxps[32 * c:32 * c + 1, :],
                                attnT[:, t:t + 1],
                                natq[t // 4][:, t % 4, c * 512:(c + 1) * 512],
                                start=(t == 0),
                                stop=(t == ET - 1),
                                skip_group_check=True,
                                tile_position=(0, 32 * c),
                            )
                    ctx_row = wkp.tile([97, 512], F32, tag="ctx_row", bufs=1,
                                       name="ctx_row_last")
                    for c in range(SC):
                        nc.scalar.activation(
                            ctx_row[32 * c:32 * c + 1, :],
                            ctxps[32 * c:32 * c + 1, :],
                            mybir.ActivationFunctionType.Copy,
                            scale=sinv_all[32 * c:32 * c + 1, :],
                        )
                        nc.sync.dma_start(
                            ctxl_d.ap()[0:1, c * 512:(c + 1) * 512],
                            ctx_row[32 * c:32 * c + 1, :],
                        )

                # batch 0's data next on the DMA queue, then the rest of W_s
                emit_load(0, split=True)
                nc.scalar.dma_start(ws_sb[:, 1:KT, :, :], ws_ap[:, 1:KT, :, :])
                emit_load(1)
                for b in range(BPC):
                    quarters, mask_st = state[b]
                    hid_prev = None
                    for k in range(KT):
                        if k == 2 and b > 0:
                            emit_energy_tail(b - 1)
                            emit_exp(b - 1)
                        if k == 3 and b > 0:
                            emit_sum(b - 1)
                        if k == 4 and b > 0:
                            emit_post_head(b - 1)
                        if k == 3 and b + 2 < BPC:
                            emit_load(b + 2)
                        if k == 5 and b < BPC - 1:
                            emit_loadB(b)
                        if k == 0 and b == BPC - 1:
                            natq = []
                            for q in range(4):
                                nq = encp.tile([P, 4, E], F8, tag="encq",
                                               name=f"natq_{q}")
                                nc.sync.dma_start(
                                    nq[:],
                                    encN_d.ap()[q].rearrange("p (t e) -> p t e", t=4),
                                )
                                natq.append(nq)
                            state["natq"] = natq
                        pss = []
                        for sc in range(SC):
                            ps = mmp.tile([P, 512], F32, tag="mm",
                                          name=f"ps_{b}_{k}_{sc}")
                            pss.append(ps)
                        for ep in range(EPAIR):
                            q, qt = divmod(2 * ep, 4)
                            for sc in range(SC):
                                nc.tensor.matmul(
                                    pss[sc][:],
                                    ws_sb[:, 2 * ep:2 * ep + 2, k * P:(k + 1) * P],
                                    quarters[q][:, qt:qt + 2, sc * 512:(sc + 1) * 512],
                                    start=(ep == 0),
                                    stop=(ep == EPAIR - 1),
                                    perf_mode=mybir.MatmulPerfMode.DoubleRow,
                                )
                        # v-chain steps for k-1's hids (DVE)
                        if hid_prev is not None:
                            for sc in range(SC):
                                vacc = wkp.tile([P, 512], BF16, tag="vacc",
                                                bufs=8, name=f"vacc_{b}_{k-1}_{sc}")
                                if k == 1:
                                    nc.vector.tensor_scalar_mul(
                                        vacc[:], hid_prev[sc][:],
                                        v_f32[:, k - 1:k])
                                else:
                                    nc.vector.scalar_tensor_tensor(
                                        vacc[:], hid_prev[sc][:],
                                        v_f32[:, k - 1:k],
                                        state[b, "vacc", sc][:],
                                        mybir.AluOpType.mult,
                                        mybir.AluOpType.add,
                                    )
                                state[b, "vacc", sc] = vacc
                        # context for the previous batch: 4 e-tiles per k
                        if b > 0 and 4 <= k <= 7:
                            emit_ctx(b - 1, range(4 * (k - 4), 4 * (k - 3)))
                        hids = []
                        for sc in range(SC):
                            hid = wkp.tile([P, 512], BF16, tag="hid", bufs=8,
                                           name=f"hid_{b}_{k}_{sc}")
                            nc.scalar.activation(
                                hid[:],
                                pss[sc][:],
                                mybir.ActivationFunctionType.Tanh,
                                bias=decf_sb[:, k, b:b + 1],
                                scale=UNSCALE,
                            )
                            hids.append(hid)
                        hid_prev = hids
                    # final v-chain steps (k = KT-1)
                    for sc in range(SC):
                        vacc = wkp.tile([P, 512], BF16, tag="vacc", bufs=8,
                                        name=f"vacc_{b}_{KT-1}_{sc}")
                        nc.vector.scalar_tensor_tensor(
                            vacc[:], hid_prev[sc][:], v_f32[:, KT - 1:KT],
                            state[b, "vacc", sc][:],
                            mybir.AluOpType.mult, mybir.AluOpType.add,
                        )
                        state[b, "vacc", sc] = vacc
                emit_energy_tail(BPC - 1)
                emit_exp(BPC - 1)
                emit_sum(BPC - 1)
                emit_post_last(BPC - 1)

    nc.compile()
    return nc


def _prep_inputs(decoder_hidden, encoder_outputs, src_mask, W_h, W_s, v):
    bf = ml_dtypes.bfloat16
    f8 = ml_dtypes.float8_e4m3

    enc = np.asarray(encoder_outputs, np.float32)          # [B, S, E]
    W_s = np.asarray(W_s, np.float32)
    v_b = np.asarray(v, np.float32).astype(bf).astype(np.float32)

    # quantized weights + linear-correction vectors
    W8 = np.clip(W_s * SCALE_W, -F8MAX, F8MAX).astype(f8)
    # device layout [p, k, t, c]: ws_dev[p, k, t, c] = W8[t*128+p, k*128+c]
    ws_dev = np.ascontiguousarray(
        W8.reshape(ET, P, KT, P).transpose(1, 2, 0, 3).reshape(P, KT * ET * P)
    )
    u = W_s @ v_b                                          # [E] exact
    u8 = W8.astype(np.float32) @ v_b                       # [E] of quantized W

    decf = np.asarray(decoder_hidden, np.float32) @ np.asarray(W_h, np.float32)

    vv = np.ascontiguousarray(v_b.astype(bf).reshape(KT, P).T)
    maskadd = np.where(np.asarray(src_mask) == 0, np.float32(NEG_BIG),
                       np.float32(0.0))

    in_maps = []
    for c in range(NC):
        lo, hi = c * BPC, (c + 1) * BPC
        encT = np.ascontiguousarray(enc[lo:hi].transpose(0, 2, 1))  # [BPC,E,S]
        encH = np.clip(encT * np.float32(SCALE_ENC), -F8MAX, F8MAX).astype(f8)
        encB = encT.astype(bf)
        # device layout [b, q, p, (qt s)]: row e = q*512 + qt*128 + p
        encH_dev = np.ascontiguousarray(
            encH.reshape(BPC, 4, 4, P, S).transpose(0, 1, 3, 2, 4)
        ).reshape(BPC, 4, P, 4 * S)
        encB_dev = np.ascontiguousarray(
            encB.reshape(BPC, 4, 4, P, S).transpose(0, 1, 3, 2, 4)
        ).reshape(BPC, 4, P, 4 * S)
        # em = exact linear term - quantized linear term + mask
        elin = enc[lo:hi] @ u                               # [BPC, S]
        elin8 = np.einsum("e,bes->bs", u8, encH.astype(np.float32),
                          optimize=True)
        em = (elin - np.float32(UNSCALE) * elin8 + maskadd[lo:hi]).astype(bf)

        dc = decf[lo:hi]                                    # [BPC, H]
        decf_arr = np.ascontiguousarray(
            dc.reshape(BPC, KT, P).transpose(2, 1, 0).reshape(P, KT * BPC)
        ).astype(np.float32)

        encN = np.clip(enc[hi - 1] * np.float32(SCALE_ENC), -F8MAX,
                       F8MAX).astype(f8)                    # [S, E] fp8
        encN_dev = np.ascontiguousarray(
            encN.reshape(4, 4, P, E).transpose(0, 2, 1, 3)
        ).reshape(4, P, 4 * E)

        in_maps.append({
            "encH": encH_dev,
            "encB": encB_dev,
            "wsT": ws_dev,
            "decf": decf_arr,
            "vv": vv,
            "emask": em,
            "encN": encN_dev,
        })
    return in_maps


def kernel(decoder_hidden, encoder_outputs, src_mask, W_h, W_s, v, _trace=False):
    if "nc" not in _CACHE:
        _CACHE["nc"] = _build()
    nc = _CACHE["nc"]

    in_maps = _prep_inputs(
        np.asarray(decoder_hidden, dtype=np.float32),
        np.asarray(encoder_outputs, dtype=np.float32),
        np.asarray(src_mask),
        np.asarray(W_h, dtype=np.float32),
        np.asarray(W_s, dtype=np.float32),
        np.asarray(v, dtype=np.float32),
    )

    res = run_bass_kernel_spmd(nc, in_maps, core_ids=list(range(NC)), trace=_trace)
    _CACHE["last_result"] = res

    context = np.empty((B, E), dtype=np.float32)
    attn = np.empty((B, S), dtype=np.float32)
    for c in range(NC):
        lo, hi = c * BPC, (c + 1) * BPC
        attn[lo:hi] = res.results[c]["attn"]
        raw = res.results[c]["ctxr"]  # [P, BPC*ET]
        context[lo:hi] = raw.reshape(P, BPC, ET).transpose(1, 2, 0).reshape(BPC, E)
        # last local batch: PE tail fast path, fp8 enc scale folded out here
        context[hi - 1] = res.results[c]["ctxl"][0] * np.float32(1.0 / SCALE_ENC)
    return context, attn
